# revision 1
# baseline (speedup 1.0000x reference)
"""Multi-head causal attention (B=2, T=2048, E=1024, H=16, D=64) on 8 TRN2 cores.

Sharding: tensor-parallel over heads. Core c owns heads {2c, 2c+1} for both
batches. Each core computes its heads' q/k/v projections, causal attention,
and a partial output projection z_c = out_c @ Wo[:, 128c:128c+128].T.
Host combines: z = sum_c z_c + bo.

Note the reference computes wei = K @ Q^T, i.e. output token t attends over
s <= t with logits k_t . q_s. We compute ST[s, t] = q_s . k_t (s on
partitions) so that the A@V matmul needs no transposes, and get the softmax
denominator via a ones-column appended to V.
"""

import numpy as np
import ml_dtypes

import concourse.bacc as bacc
import concourse.mybir as mybir
import concourse.tile as tile
from concourse.bass_utils import run_bass_kernel_spmd
from concourse.masks import make_identity


def _make_runner(nc):
    """Persistent jitted SPMD callable (avoids per-call jit re-trace)."""
    import jax
    from jax.sharding import Mesh, NamedSharding, PartitionSpec
    try:
        from jax.experimental.shard_map import shard_map
    except ImportError:
        shard_map = jax.shard_map
    from concourse.bass2jax import (_bass_exec_p, install_neuronx_cc_hook,
                                    partition_id_tensor)

    install_neuronx_cc_hook()
    partition_name = (nc.partition_id_tensor.name
                      if nc.partition_id_tensor else None)
    in_names, out_names, out_avals, zero_outs = [], [], [], []
    for alloc in nc.m.functions[0].allocations:
        if not isinstance(alloc, mybir.MemoryLocationSet):
            continue
        name = alloc.memorylocations[0].name
        if alloc.kind == "ExternalInput":
            if name != partition_name:
                in_names.append(name)
        elif alloc.kind == "ExternalOutput":
            shape = tuple(alloc.tensor_shape)
            dtype = mybir.dt.np(alloc.dtype)
            out_names.append(name)
            out_avals.append(jax.core.ShapedArray(shape, dtype))
            zero_outs.append(np.zeros(shape, dtype))
    n_params = len(in_names)
    all_in = list(in_names) + list(out_names)
    if partition_name is not None:
        all_in.append(partition_name)

    def _body(*args):
        operands = list(args)
        if partition_name is not None:
            operands.append(partition_id_tensor())
        return tuple(_bass_exec_p.bind(
            *operands, out_avals=tuple(out_avals), in_names=tuple(all_in),
            out_names=tuple(out_names), lowering_input_output_aliases=(),
            sim_require_finite=True, sim_require_nnan=True, nc=nc))

    devices = jax.devices()[:N_CORES]
    mesh = Mesh(np.asarray(devices), ("core",))
    spec = NamedSharding(mesh, PartitionSpec("core"))
    rspec = NamedSharding(mesh, PartitionSpec())
    # inputs identical on every core are sent once and replicated
    replicated = {"xt", "mask"}
    in_specs = tuple(
        (PartitionSpec() if nm in replicated else PartitionSpec("core"))
        for nm in in_names) + (PartitionSpec("core"),) * len(out_names)
    fn = jax.jit(
        shard_map(_body, mesh=mesh, in_specs=in_specs,
                  out_specs=(PartitionSpec("core"),) * len(out_names),
                  check_rep=False),
        keep_unused=True)
    zeros_dev = [
        jax.device_put(np.zeros((N_CORES * z.shape[0], *z.shape[1:]), z.dtype),
                       spec) for z in zero_outs
    ]

    def run(in_maps):
        concat = [
            jax.device_put(np.asarray(in_maps[0][nm]), rspec)
            if nm in replicated else
            jax.device_put(
                np.concatenate([np.asarray(in_maps[c][nm])
                                for c in range(N_CORES)], axis=0), spec)
            for nm in in_names
        ]
        outs = fn(*concat, *zeros_dev)
        fulls = [np.asarray(outs[i]).reshape(N_CORES, *out_avals[i].shape)
                 for i in range(len(out_names))]
        return [{nm: fulls[i][c] for i, nm in enumerate(out_names)}
                for c in range(N_CORES)]

    return run

N_CORES = 8
B, T, E = 2, 2048, 1024
H, D = 16, 64
HPC = H // N_CORES          # heads per core = 2
F = HPC * D                 # local feature cols = 128
TBLK = 512                  # t-block width for stage A
NTB = T // TBLK             # 4
NSC = T // 128              # s-chunks = 16
NEC = E // 128              # e-chunks = 8
EXP_BIAS = -2.0             # exp(S + EXP_BIAS); cancels in softmax, guards overflow

F32 = mybir.dt.float32
F16 = mybir.dt.float16
F32R = mybir.dt.float32r
BF16 = mybir.dt.bfloat16
EXP = mybir.ActivationFunctionType.Exp


def build_nc(rep=1, cfg=None):
    cfg = dict(cfg or {})
    any_copy = cfg.get("any_copy", False)
    evict = cfg.get("evict", "mixed")  # mixed|zscalar|zvector
    sp_bufs = cfg.get("sp_bufs", 2)
    op_bufs = cfg.get("op_bufs", 2)
    misc_bufs = cfg.get("misc_bufs", None)  # if set, tp+zp merged [128,512] x misc_bufs
    pt_bufs = cfg.get("pt_bufs", 4)
    out_q = cfg.get("out_q", "scalar")  # engine for output DMAs
    xt_bf16 = cfg.get("xt_bf16", False)
    skip_z = cfg.get("skip_z", False)
    skip_b = cfg.get("skip_b", False)
    skip_attn = cfg.get("skip_attn", False)
    td_form = cfg.get("td_form", False)
    sp_wide = cfg.get("sp_wide", False)
    nc = bacc.Bacc("TRN2", target_bir_lowering=False, debug=False,
                   num_devices=N_CORES)

    xt = nc.dram_tensor("xt", [B, E, T], BF16 if xt_bf16 else F32R,
                        kind="ExternalInput").ap()
    wq = nc.dram_tensor("wq", [E, F], BF16 if xt_bf16 else F32R, kind="ExternalInput").ap()
    wk = nc.dram_tensor("wk", [E, F], BF16 if xt_bf16 else F32R, kind="ExternalInput").ap()
    wv = nc.dram_tensor("wv", [E, F], BF16 if xt_bf16 else F32R, kind="ExternalInput").ap()
    wot = nc.dram_tensor("wot", [F, E], F32R, kind="ExternalInput").ap()
    mask = nc.dram_tensor("mask", [128, 128], BF16, kind="ExternalInput").ap()
    zp = nc.dram_tensor("zp", [B, T, E], F16, kind="ExternalOutput").ap()

    with tile.TileContext(nc) as tc:
        with (
            tc.tile_pool(name="const", bufs=1) as cpool,
            tc.tile_pool(name="xtp", bufs=36) as xtp,
            tc.tile_pool(name="proj", bufs=2) as projp,
            tc.tile_pool(name="v2p", bufs=2 * NSC) as v2p,
            tc.tile_pool(name="ptp", bufs=pt_bufs) as ptp,
            tc.tile_pool(name="smallp", bufs=4) as smallp,
            tc.tile_pool(name="zsbp", bufs=3) as zsbp,
            tc.tile_pool(name="ps_s", bufs=sp_bufs, space="PSUM") as ps_s,
            tc.tile_pool(name="ps_o", bufs=op_bufs, space="PSUM") as ps_o,
            tc.tile_pool(name="ps_t", bufs=(misc_bufs or 2), space="PSUM") as ps_t,
        ):
            # ---- constants (loaded once) ----
            ident = cpool.tile([128, 128], F32, tag="ident")
            make_identity(nc, ident[:])
            mask_sb = cpool.tile([128, 128], BF16, tag="mask")
            nc.scalar.dma_start(mask_sb[:], mask)
            ebias = cpool.tile([128, 1], F32, tag="ebias")
            nc.vector.memset(ebias[:], EXP_BIAS)
            # one coalesced DMA per weight tensor: [E, F] -> [128, NEC*F]
            wq_sb = []
            wk_sb = []
            wv_sb = []
            for lst, wsrc, nm in ((wq_sb, wq, "wq"), (wk_sb, wk, "wk"),
                                  (wv_sb, wv, "wv")):
                t_ = cpool.tile([128, NEC * F], BF16 if xt_bf16 else F32R,
                                tag=f"wall{nm}")
                nc.scalar.dma_start(
                    t_.rearrange("p (a c) -> p a c", a=NEC),
                    wsrc.rearrange("(a p) c -> p a c", p=128))
                for e in range(NEC):
                    lst.append(t_[:, e * F:(e + 1) * F])
            wot_sb = cpool.tile([F, E], F32R, tag="wot")
            nc.scalar.dma_start(wot_sb[:], wot)

            def body():
                for b in range(B):
                    # ---- load transposed activations (t-halves so the first
                    # projection group can start after half the input DMA) ----
                    xth = [[None] * 4 for _ in range(NEC)]
                    for qt in range(4):
                        for e in range(NEC):
                            t_ = xtp.tile([128, T // 4],
                                          BF16 if xt_bf16 else F32R, tag="xt")
                            nc.sync.dma_start(
                                t_[:], xt[b, e * 128:(e + 1) * 128,
                                          qt * (T // 4):(qt + 1) * (T // 4)])
                            xth[e][qt] = t_

                    # ---- projections: qT2/kT2/vT2 [128(f), T] ----
                    heads = {}
                    for nm, wsb in (("q", wq_sb), ("k", wk_sb), ("v", wv_sb)):
                        dst = projp.tile([128, T], F32R if nm != "v" else F32, tag=f"{nm}T2")
                        for tp2 in range(T // 1024):
                            ps = ps_s.tile([128, 1024], F32, tag="sp")
                            for half in range(2):
                                c0 = tp2 * 1024 + half * 512
                                for e in range(NEC):
                                    nc.tensor.matmul(
                                        ps[:, half * 512:(half + 1) * 512],
                                        wsb[e],
                                        xth[e][c0 // 512][:],
                                        start=(e == 0), stop=(e == NEC - 1))
                            (nc.any if any_copy else nc.vector).tensor_copy(
                                dst[:, tp2 * 1024:(tp2 + 1) * 1024], ps[:])
                        heads[nm] = dst
                    qT2, kT2, vT2 = heads["q"], heads["k"], heads["v"]

                    # ---- v2[s]: [128(s), 130] bf16 = [1|v_h0|1|v_h1] ----
                    v2 = []
                    for s in range(NSC):
                        tpw = ps_t.tile([128, 512], F32, tag="tp")
                        tp_ = tpw[:, 0:128]
                        nc.tensor.matmul(tp_[:], vT2[:, s * 128:(s + 1) * 128],
                                         ident[:], is_transpose=True)
                        v2t = v2p.tile([128, 130], BF16, tag="v2")
                        v2r = v2t.rearrange("p (g c) -> p g c", g=2)
                        nc.vector.memset(v2r[:, :, 64:65], 1.0)
                        nc.vector.tensor_copy(
                            v2r[:, :, 0:64],
                            tp_.rearrange("p (g c) -> p g c", g=2))
                        v2.append(v2t)

                    # ---- attention (stage B in outT form) ----
                    for tb in range(NTB if not skip_attn else 0):
                        slast = 4 * tb + 3
                        po = {}
                        for h in range(2 if not skip_b else 0):
                            if td_form:
                                po_t = ps_o.tile([128, 260], F32, tag="op")
                            else:
                                po_t = ps_o.tile([65, 512], F32, tag="op")
                            po[h] = po_t
                        npairs = 2 * tb + 2
                        for p in range(npairs):
                            pts = []
                            for h in range(2):
                                ps = ps_s.tile([128, 1024], F32, tag="sp")
                                for dp in range(2):
                                    si = 2 * p + dp
                                    r = si - 4 * tb
                                    # trim causally-dead columns where fp32r
                                    # still streams at 1 cyc/row (N >= 256)
                                    c0 = 128 * r if r in (1, 2) else 0
                                    nc.tensor.matmul(
                                        ps[:, dp * 512 + c0:(dp + 1) * 512],
                                        qT2[64 * h:64 * h + 64,
                                            si * 128:(si + 1) * 128],
                                        kT2[64 * h:64 * h + 64,
                                            tb * 512 + c0:(tb + 1) * 512],
                                        start=True, stop=True)
                                pt = ptp.tile([128, 1024], BF16, tag="pt")
                                nc.scalar.activation(pt[:], ps[:], EXP,
                                                     bias=ebias[:])
                                for dp in range(2):
                                    si = 2 * p + dp
                                    r = si - 4 * tb
                                    if 0 <= r < 4:
                                        sl = pt[:, dp * 512 + r * 128:
                                                dp * 512 + (r + 1) * 128]
                                        meng = (nc.gpsimd if cfg.get("mask_pool")
                                                else nc.vector)
                                        meng.tensor_mul(sl, sl, mask_sb[:])
                                pts.append(pt)
                            for dp in range(2 if not skip_b else 0):
                                si = 2 * p + dp
                                for h in range(2):
                                    if td_form:
                                        # po[h] is [128(t), 260]: j-th chunk at
                                        # cols 130*(j%2); heads share tile pair
                                        for j in range(4):
                                            tcg = 4 * tb + j
                                            if si > tcg:
                                                continue
                                            jj = j % 2
                                            dst = po[j // 2][:, jj * 130 + h * 65:
                                                             jj * 130 + (h + 1) * 65]
                                            nc.tensor.matmul(
                                                dst,
                                                pts[h][:, dp * 512 + j * 128:
                                                       dp * 512 + (j + 1) * 128],
                                                v2[si][:, h * 65:(h + 1) * 65],
                                                start=(si == 0 and h == 0),
                                                stop=(si == tcg),
                                                skip_group_check=True)
                                    else:
                                        r = si - 4 * tb
                                        c0 = max(r, 0) * 128
                                        nc.tensor.matmul(
                                            po[h][:, c0:512],
                                            v2[si][:, h * 65:(h + 1) * 65],
                                            pts[h][:, dp * 512 + c0:
                                                   (dp + 1) * 512],
                                            start=(si == 0), stop=(si == slast),
                                            skip_group_check=True)

                        # ---- normalize (rows 1:65 / row 0) + partial z ----
                        if not skip_b and not skip_z:
                            outT = smallp.tile([128, 512], F32R, tag="outT")
                            for h in range(2):
                                rrow = smallp.tile([1, 512], F32, tag="rrow")
                                nc.vector.reciprocal(rrow[:], po[h][64:65, :])
                                rbc = smallp.tile([64, 512], F32, tag="rbc")
                                nc.gpsimd.partition_broadcast(rbc[:], rrow[:])
                                nc.vector.tensor_mul(
                                    outT[64 * h:64 * h + 64, :],
                                    po[h][0:64, :], rbc[:])
                            for jp2 in range(2):
                                zsb = zsbp.tile([128, 2048], F16, tag="zsb")
                                for jj in range(2):
                                    j = 2 * jp2 + jj
                                    for eb in range(2):
                                        zps = ps_t.tile([128, 512], F32,
                                                        tag="tp")
                                        nc.tensor.matmul(
                                            zps[:],
                                            outT[:, j * 128:(j + 1) * 128],
                                            wot_sb[:, eb * 512:(eb + 1) * 512],
                                            start=True, stop=True)
                                        dstsl = zsb[:, jj * 1024 + eb * 512:
                                                    jj * 1024 + (eb + 1) * 512]
                                        if evict == "zscalar":
                                            nc.scalar.copy(dstsl, zps[:])
                                        elif evict == "zvector":
                                            nc.vector.tensor_copy(dstsl, zps[:])
                                        elif any_copy:
                                            nc.any.tensor_copy(dstsl, zps[:])
                                        elif eb == 0:
                                            nc.vector.tensor_copy(dstsl, zps[:])
                                        else:
                                            nc.scalar.copy(dstsl, zps[:])
                                t0r = (4 * tb + 2 * jp2) * 128
                                getattr(nc, out_q).dma_start(
                                    zp[b, t0r:t0r + 256, :]
                                    .rearrange("(a p) c -> p a c", p=128),
                                    zsb.rearrange("p (a c) -> p a c", a=2))

            if rep == 1:
                body()
            else:
                with tc.For_i(0, rep, 1):
                    body()

    nc.compile()
    return nc


def make_in_maps(inputs, Wk, Wq, Wv, Wo, xt_bf16=False):
    """Shard full inputs into per-core input maps."""
    wdt = ml_dtypes.bfloat16 if xt_bf16 else np.float32
    xt = np.ascontiguousarray(inputs.transpose(0, 2, 1)).astype(wdt)
    scale = np.float32(D ** -0.5)
    tri = (np.arange(128)[None, :] >= np.arange(128)[:, None])
    mask = tri.astype(ml_dtypes.bfloat16)
    in_maps = []
    for c in range(N_CORES):
        h0 = HPC * c
        wq2 = np.ascontiguousarray(
            np.concatenate([Wq[h0 + i] for i in range(HPC)], axis=1))
        wk2 = np.ascontiguousarray(
            np.concatenate([Wk[h0 + i] for i in range(HPC)], axis=1)) * scale
        wv2 = np.ascontiguousarray(
            np.concatenate([Wv[h0 + i] for i in range(HPC)], axis=1))
        wot = np.ascontiguousarray(Wo[:, F * c:F * (c + 1)].T)
        in_maps.append({
            "xt": xt,
            "wq": wq2.astype(wdt),
            "wk": wk2.astype(wdt),
            "wv": wv2.astype(wdt),
            "wot": wot.astype(np.float32),
            "mask": mask,
        })
    return in_maps


_NC = None
_RUN = None
DEFAULT_CFG = {"any_copy": True, "out_q": "sync", "xt_bf16": False}


def kernel(inputs, Wk, Wq, Wv, Wo, bo):
    global _NC, _RUN
    if _NC is None:
        _NC = build_nc(cfg=DEFAULT_CFG)
    in_maps = make_in_maps(inputs, Wk, Wq, Wv, Wo,
                           xt_bf16=DEFAULT_CFG["xt_bf16"])
    try:
        if _RUN is None:
            _RUN = _make_runner(_NC)
        results = _RUN(in_maps)
    except Exception:
        _RUN = False if _RUN is None else _RUN
        res = run_bass_kernel_spmd(_NC, in_maps,
                                   core_ids=list(range(N_CORES)))
        results = res.results
    z = np.zeros((B, T, E), dtype=np.float32)
    for c in range(N_CORES):
        z += results[c]["zp"].astype(np.float32)
    return z + bo.astype(np.float32)



# revision 22
# speedup vs baseline: 1.1057x; 1.1057x over previous
"""Multi-head causal attention (B=2, T=2048, E=1024, H=16, D=64) on 8 TRN2 cores.

Sharding: tensor-parallel over heads. Core c owns heads {2c, 2c+1} for both
batches. Each core computes its heads' q/k/v projections, causal attention,
and a partial output projection z_c = out_c @ Wo[:, 128c:128c+128].T.
Host combines: z = sum_c z_c + bo.

v2 design:
- Q/K projections in fp8e4 DoubleRow (contraction 256/matmul), V in bf16.
  Weights prescaled x64 for fp8 range; k eviction rescales by 1/(64*64*8).
- Scores ST[s, t] = q_s . k_t computed per (pair-of-s-chunks, head) with
  pair-shared causal windows; causal mask applied on the PE as an extra
  accumulating matmul adding -30 to dead entries (maskL stationary,
  maskRc moving), so exp feeds A@V directly.
- exp on Act engine only (with EXP_BIAS), output fp8 (bf16 for tb=0).
- A@V in fp8 DoubleRow over s-chunk pairs; tb=0 in bf16 so early tokens
  avoid fp8 value noise. Ones column in V gives the softmax denominator.
- Emission is software-pipelined (AV lags QK by LAG units) and the two
  batches' attention blocks are interleaved to balance PE vs Act.
"""

import numpy as np
import ml_dtypes

import concourse.bacc as bacc
import concourse.mybir as mybir
import concourse.tile as tile
from concourse.bass_utils import run_bass_kernel_spmd


def _make_runner(nc):
    """Persistent jitted SPMD callable (avoids per-call jit re-trace)."""
    import jax
    from jax.sharding import Mesh, NamedSharding, PartitionSpec
    try:
        from jax.experimental.shard_map import shard_map
    except ImportError:
        shard_map = jax.shard_map
    from concourse.bass2jax import (_bass_exec_p, install_neuronx_cc_hook,
                                    partition_id_tensor)

    install_neuronx_cc_hook()
    partition_name = (nc.partition_id_tensor.name
                      if nc.partition_id_tensor else None)
    in_names, out_names, out_avals, zero_outs = [], [], [], []
    for alloc in nc.m.functions[0].allocations:
        if not isinstance(alloc, mybir.MemoryLocationSet):
            continue
        name = alloc.memorylocations[0].name
        if alloc.kind == "ExternalInput":
            if name != partition_name:
                in_names.append(name)
        elif alloc.kind == "ExternalOutput":
            shape = tuple(alloc.tensor_shape)
            dtype = mybir.dt.np(alloc.dtype)
            out_names.append(name)
            out_avals.append(jax.core.ShapedArray(shape, dtype))
            zero_outs.append(np.zeros(shape, dtype))
    all_in = list(in_names) + list(out_names)
    if partition_name is not None:
        all_in.append(partition_name)

    def _body(*args):
        operands = list(args)
        if partition_name is not None:
            operands.append(partition_id_tensor())
        return tuple(_bass_exec_p.bind(
            *operands, out_avals=tuple(out_avals), in_names=tuple(all_in),
            out_names=tuple(out_names), lowering_input_output_aliases=(),
            sim_require_finite=True, sim_require_nnan=True, nc=nc))

    devices = jax.devices()[:N_CORES]
    mesh = Mesh(np.asarray(devices), ("core",))
    spec = NamedSharding(mesh, PartitionSpec("core"))
    rspec = NamedSharding(mesh, PartitionSpec())
    in_specs = tuple(
        (PartitionSpec() if nm in REPLICATED else PartitionSpec("core"))
        for nm in in_names) + (PartitionSpec("core"),) * len(out_names)
    fn = jax.jit(
        shard_map(_body, mesh=mesh, in_specs=in_specs,
                  out_specs=(PartitionSpec("core"),) * len(out_names),
                  check_rep=False),
        keep_unused=True)
    zeros_dev = [
        jax.device_put(np.zeros((N_CORES * z.shape[0], *z.shape[1:]), z.dtype),
                       spec) for z in zero_outs
    ]

    def run(in_maps):
        concat = [
            jax.device_put(np.asarray(in_maps[0][nm]), rspec)
            if nm in REPLICATED else
            jax.device_put(
                np.concatenate([np.asarray(in_maps[c][nm])
                                for c in range(N_CORES)], axis=0), spec)
            for nm in in_names
        ]
        outs = fn(*concat, *zeros_dev)
        fulls = [np.asarray(outs[i]).reshape(N_CORES, *out_avals[i].shape)
                 for i in range(len(out_names))]
        return [{nm: fulls[i][c] for i, nm in enumerate(out_names)}
                for c in range(N_CORES)]

    return run

N_CORES = 8
B, T, E = 2, 2048, 1024
H, D = 16, 64
HPC = H // N_CORES          # heads per core = 2
F = HPC * D                 # local feature cols = 128
NTB = T // 512              # 4 t-blocks
NSC = T // 128              # 16 s-chunks
WS = 64.0                   # fp8 weight prescale
KSCALE = 1.0 / (WS * WS * 8.0)  # folded into k eviction (incl D^-0.5)
EXP_BIAS = -2.0
NEG = -30.0

REPLICATED = {"xb", "x8", "identb", "maskL", "maskRc"}

F32 = mybir.dt.float32
F16 = mybir.dt.float16
F32R = mybir.dt.float32r
BF16 = mybir.dt.bfloat16
FP8 = mybir.dt.float8e4
EXP = mybir.ActivationFunctionType.Exp
DR = mybir.MatmulPerfMode.DoubleRow


def build_nc(rep=1, cfg=None):
    cfg = dict(cfg or {})
    lag = cfg.get("lag", 2)
    nc = bacc.Bacc("TRN2", target_bir_lowering=False, debug=False,
                   num_devices=N_CORES)

    xb = nc.dram_tensor("xb", [B, E, T], BF16, kind="ExternalInput").ap()
    x8 = nc.dram_tensor("x8", [B, E, T], FP8, kind="ExternalInput").ap()
    wq8 = nc.dram_tensor("wq8", [E, F], FP8, kind="ExternalInput").ap()
    wk8 = nc.dram_tensor("wk8", [E, F], FP8, kind="ExternalInput").ap()
    wv = nc.dram_tensor("wv", [E, F], BF16, kind="ExternalInput").ap()
    wot = nc.dram_tensor("wot", [F, E], F32R, kind="ExternalInput").ap()
    identb = nc.dram_tensor("identb", [128, 128], F32,
                            kind="ExternalInput").ap()
    maskL = nc.dram_tensor("maskL", [128, 128], BF16,
                           kind="ExternalInput").ap()
    maskRc = nc.dram_tensor("maskRc", [128, 256], BF16,
                            kind="ExternalInput").ap()
    zp = nc.dram_tensor("zp", [B, T, E], F16, kind="ExternalOutput").ap()

    with tile.TileContext(nc) as tc:
        with (
            tc.tile_pool(name="const", bufs=1) as cpool,
            tc.tile_pool(name="xbp", bufs=32) as xbp,
            tc.tile_pool(name="x8p", bufs=16) as x8p,
            tc.tile_pool(name="proj", bufs=4) as projp,
            tc.tile_pool(name="v2p", bufs=32) as v2p,
            tc.tile_pool(name="v2bp", bufs=8) as v2bp,
            tc.tile_pool(name="ptp", bufs=4) as ptp,
            tc.tile_pool(name="smallp", bufs=4) as smallp,
            tc.tile_pool(name="zsbp", bufs=3) as zsbp,
            tc.tile_pool(name="ps_s", bufs=2, space="PSUM") as ps_s,
            tc.tile_pool(name="ps_o", bufs=2, space="PSUM") as ps_o,
            tc.tile_pool(name="ps_t", bufs=2, space="PSUM") as ps_t,
        ):
            # ---- constants (loaded once) ----
            ident = cpool.tile([128, 128], F32, tag="ident")
            nc.scalar.dma_start(ident[:], identb)
            mL = cpool.tile([128, 128], BF16, tag="mL")
            nc.scalar.dma_start(mL[:], maskL)
            mRc = cpool.tile([128, 256], BF16, tag="mRc")
            nc.scalar.dma_start(mRc[:], maskRc)
            ebias = cpool.tile([128, 1], F32, tag="ebias")
            nc.vector.memset(ebias[:], EXP_BIAS)
            wq_sb = cpool.tile([128, 2 * E // 2], FP8, tag="wq")  # [128,(c j f)]
            wk_sb = cpool.tile([128, 2 * E // 2], FP8, tag="wk")
            for t_, src in ((wq_sb, wq8), (wk_sb, wk8)):
                nc.scalar.dma_start(
                    t_.rearrange("p (c j f) -> p c j f", c=4, j=2),
                    src.rearrange("(c j p) f -> p c j f", j=2, p=128))
            wv_sb = cpool.tile([128, E], BF16, tag="wv")  # [128, (c f)]
            nc.scalar.dma_start(
                wv_sb.rearrange("p (c f) -> p c f", c=8),
                wv.rearrange("(c p) f -> p c f", p=128))
            wot_sb = cpool.tile([F, E], F32R, tag="wot")
            nc.scalar.dma_start(wot_sb[:], wot)
            wqr = wq_sb.rearrange("p (c j f) -> p c j f", c=4, j=2)
            wkr = wk_sb.rearrange("p (c j f) -> p c j f", c=4, j=2)
            wvr = wv_sb.rearrange("p (c f) -> p c f", c=8)

            def body():
                st = {}

                def emit_loads(b):
                    xbt = [[None] * 2 for _ in range(8)]   # [ec][half]
                    x8t = [[None] * 2 for _ in range(4)]   # [c][half]
                    for half in range(2):
                        for ec in range(8):
                            t_ = xbp.tile([128, 1024], BF16, tag="xb")
                            nc.sync.dma_start(
                                t_[:], xb[b, ec * 128:(ec + 1) * 128,
                                          half * 1024:(half + 1) * 1024])
                            xbt[ec][half] = t_
                        for c in range(4):
                            t_ = x8p.tile([128, 2, 1024], FP8, tag="x8")
                            nc.scalar.dma_start(
                                t_[:],
                                x8[b, c * 256:(c + 1) * 256,
                                   half * 1024:(half + 1) * 1024]
                                .rearrange("(j p) t -> p j t", j=2))
                            x8t[c][half] = t_
                    st[b] = {"xbt": xbt, "x8t": x8t}

                def emit_proj(b):
                    xbt, x8t = st[b]["xbt"], st[b]["x8t"]
                    qT2, kT2, vT2 = [], [], []
                    for half in range(2):
                        qh = projp.tile([128, T // 2], BF16, tag="qT2",
                                        name=f"q{b}{half}")
                        kh = projp.tile([128, T // 2], BF16, tag="kT2",
                                        name=f"k{b}{half}")
                        vh = projp.tile([128, T // 2], F32, tag="vT2",
                                        name=f"v{b}{half}")
                        qT2.append(qh)
                        kT2.append(kh)
                        vT2.append(vh)
                        for nm in ("q", "k", "v"):
                            ps = ps_s.tile([128, 1024], F32, tag="sp")
                            for sub in range(2):
                                sl = ps[:, sub * 512:(sub + 1) * 512]
                                if nm == "v":
                                    for ec in range(8):
                                        nc.tensor.matmul(
                                            sl, wvr[:, ec],
                                            xbt[ec][half][:, sub * 512:
                                                          (sub + 1) * 512],
                                            start=(ec == 0), stop=(ec == 7))
                                else:
                                    w = wqr if nm == "q" else wkr
                                    for c in range(4):
                                        nc.tensor.matmul(
                                            sl, w[:, c],
                                            x8t[c][half][:, :,
                                                         sub * 512:
                                                         (sub + 1) * 512],
                                            start=(c == 0), stop=(c == 3),
                                            perf_mode=DR)
                            if nm == "q":
                                nc.vector.tensor_copy(qh[:], ps[:])
                            elif nm == "k":
                                nc.vector.tensor_scalar_mul(
                                    kh[:], ps[:], KSCALE)
                            else:
                                nc.vector.tensor_copy(vh[:], ps[:])
                    st[b].update(qT2=qT2, kT2=kT2, vT2=vT2)

                def emit_v2(b):
                    vT2 = st[b]["vT2"]
                    v2dr = []                 # [pp][h]: [128, (j d65)] fp8
                    v2b = []                  # [si<4]: [128, (g d65)] bf16
                    for pp in range(8):
                        tpw = ps_t.tile([128, 512], F32, tag="tp")
                        for j in range(2):
                            si = 2 * pp + j
                            nc.tensor.matmul(
                                tpw[:, j * 128:(j + 1) * 128],
                                vT2[si // 8][:, (si % 8) * 128:
                                             (si % 8 + 1) * 128],
                                ident[:], is_transpose=True)
                        tpr = tpw[:, 0:256].rearrange("p (j g d) -> p j g d",
                                                      j=2, g=2)
                        pair = []
                        for h in range(2):
                            # j-stride 80 (= 16*5) satisfies the dual-fp8
                            # ldweights step%16==0 restriction; cols 65:80
                            # are zero padding (po rows 65:80 unused)
                            t8 = v2p.tile([128, 160], FP8, tag="v2dr",
                                          name=f"v2dr{pp}{h}")
                            r8 = t8.rearrange("p (j d) -> p j d", j=2)
                            nc.vector.memset(r8[:, :, 64:80], 0.0)
                            nc.vector.memset(r8[:, :, 64:65], 1.0)
                            nc.vector.tensor_copy(r8[:, :, 0:64],
                                                  tpr[:, :, h])
                            pair.append(t8)
                        v2dr.append(pair)
                        if pp < 2:
                            for j in range(2):
                                si = 2 * pp + j
                                tb16 = v2bp.tile([128, 130], BF16, tag="v2b")
                                rb = tb16.rearrange("p (g d) -> p g d", g=2)
                                nc.vector.memset(rb[:, :, 64:65], 1.0)
                                nc.vector.tensor_copy(
                                    rb[:, :, 0:64], tpr[:, j])
                                v2b.append(tb16)
                    st[b].update(v2dr=v2dr, v2b=v2b)

                # ---- attention as a flat pipelined unit stream ----
                def attn_units(b, tb):
                    """Yield emit-closures; caller pipelines QK vs AV."""
                    qT2, kT2 = st[b]["qT2"], st[b]["kT2"]
                    v2dr, v2b = st[b]["v2dr"], st[b]["v2b"]
                    npairs = 2 * tb + 2
                    po = {}

                    def qk(p, h):
                        c0 = 256 if (p == 2 * tb + 1) else 0
                        ps = ps_s.tile([128, 1024], F32, tag="sp")
                        for dp in range(2):
                            si = 2 * p + dp
                            r = si - 4 * tb
                            has_mask = 0 <= r < 4
                            nc.tensor.matmul(
                                ps[:, dp * 512 + c0:(dp + 1) * 512],
                                qT2[si // 8][64 * h:64 * h + 64,
                                             (si % 8) * 128:
                                             (si % 8 + 1) * 128],
                                kT2[tb // 2][64 * h:64 * h + 64,
                                             (tb % 2) * 512 + c0:
                                             (tb % 2 + 1) * 512],
                                start=True, stop=not has_mask,
                                skip_group_check=True)
                            if has_mask:
                                # -30 into dead cols: [full-dead 128 |
                                # tri 128] contiguous in maskRc for odd r,
                                # tri-only for even r
                                if r in (1, 3):
                                    lo, mr = (r - 1) * 128, mRc[:, 0:256]
                                else:
                                    lo, mr = r * 128, mRc[:, 128:256]
                                nc.tensor.matmul(
                                    ps[:, dp * 512 + lo:
                                       dp * 512 + r * 128 + 128],
                                    mL[:], mr,
                                    start=False, stop=True,
                                    skip_group_check=True)
                        # exp -> pt
                        if tb == 0:
                            pt = ptp.tile([128, 1024], BF16, tag="ptb")
                        else:
                            pt = ptp.tile([128, 1024], FP8, tag="pt8")
                        if c0 == 0:
                            nc.scalar.activation(pt[:], ps[:], EXP,
                                                 bias=ebias[:])
                        else:
                            pr = pt.rearrange("p (j t) -> p j t", j=2)
                            sr = ps.rearrange("p (j t) -> p j t", j=2)
                            nc.scalar.activation(pr[:, :, c0:512],
                                                 sr[:, :, c0:512], EXP,
                                                 bias=ebias[:])
                        return pt

                    def av(p, h, pt):
                        if h not in po:
                            po[h] = ps_o.tile([80, 512], F32, tag="op",
                                              name=f"po{h}")
                        c0 = 256 if (p == 2 * tb + 1) else 0
                        if tb == 0:
                            for dp in range(2):
                                si = 2 * p + dp
                                cs = si * 128
                                nc.tensor.matmul(
                                    po[h][0:65, cs:512],
                                    v2b[si][:, h * 65:(h + 1) * 65],
                                    pt[:, dp * 512 + cs:(dp + 1) * 512],
                                    start=(si == 0), stop=(si == 3),
                                    skip_group_check=True)
                        else:
                            pr = pt.rearrange("p (j t) -> p j t", j=2)
                            nc.tensor.matmul(
                                po[h][:, c0:512],
                                v2dr[p][h].rearrange("p (j d) -> p j d", j=2),
                                pr[:, :, c0:512],
                                start=(p == 0), stop=(p == npairs - 1),
                                perf_mode=DR, skip_group_check=True)

                    units = [(p, h) for p in range(npairs) for h in range(2)]
                    for u in units:
                        yield ("qk", b, tb, u, qk, av, po)

                def finish_block(b, tb, po):
                    # normalize + partial z projection + output DMA
                    outT = smallp.tile([128, 512], F32R, tag="outT")
                    for h in range(2):
                        rrow = smallp.tile([1, 512], F32, tag="rrow")
                        nc.vector.reciprocal(rrow[:], po[h][64:65, :])
                        rbc = smallp.tile([64, 512], F32, tag="rbc")
                        nc.gpsimd.partition_broadcast(rbc[:], rrow[:])
                        nc.vector.tensor_mul(outT[64 * h:64 * h + 64, :],
                                             po[h][0:64, :], rbc[:])
                    for jp2 in range(2):
                        zsb = zsbp.tile([128, 2048], F16, tag="zsb")
                        for jj in range(2):
                            j = 2 * jp2 + jj
                            for eb in range(2):
                                zps = ps_t.tile([128, 512], F32, tag="tp")
                                nc.tensor.matmul(
                                    zps[:], outT[:, j * 128:(j + 1) * 128],
                                    wot_sb[:, eb * 512:(eb + 1) * 512],
                                    start=True, stop=True)
                                dstsl = zsb[:, jj * 1024 + eb * 512:
                                            jj * 1024 + (eb + 1) * 512]
                                if eb == 1 and jj == 0 and jp2 == 0:
                                    nc.scalar.copy(dstsl, zps[:])
                                else:
                                    nc.vector.tensor_copy(dstsl, zps[:])
                        t0r = (4 * tb + 2 * jp2) * 128
                        nc.sync.dma_start(
                            zp[b, t0r:t0r + 256, :]
                            .rearrange("(a p) c -> p a c", p=128),
                            zsb.rearrange("p (a c) -> p a c", a=2))

                # ---- master emission ----
                emit_loads(0)
                emit_proj(0)
                emit_v2(0)
                emit_loads(1)

                # interleave order of attention blocks
                order = [(0, 0), (0, 1), (1, 0), (0, 2), (1, 1), (0, 3),
                         (1, 2), (1, 3)]
                # injected non-attention work after block i of the stream
                inject = {0: lambda: emit_proj(1), 1: lambda: emit_v2(1)}

                # flat pipeline across all blocks
                pending = []        # (b, tb, u, av, pt, po, is_last)
                bi = 0
                for (b, tb) in order:
                    gen = list(attn_units(b, tb))
                    n = len(gen)
                    for i, (_, bb, tt, u, qk, av, po) in enumerate(gen):
                        pt = qk(*u)
                        pending.append((bb, tt, u, av, pt, po, i == n - 1))
                        if len(pending) > lag:
                            _flush_one(pending, finish_block)
                    if bi in inject:
                        # drain so injected proj uses ps_s safely interleaved
                        inject[bi]()
                    bi += 1
                while pending:
                    _flush_one(pending, finish_block)

            def _flush_one(pending, finish_block):
                bb, tt, u, av, pt, po, last = pending.pop(0)
                av(*u, pt)
                if last:
                    finish_block(bb, tt, po)

            if rep == 1:
                body()
            else:
                with tc.For_i(0, rep, 1):
                    body()

    nc.compile()
    return nc


def make_in_maps(inputs, Wk, Wq, Wv, Wo):
    """Shard full inputs into per-core input maps."""
    xt = np.ascontiguousarray(inputs.transpose(0, 2, 1))
    xbn = xt.astype(ml_dtypes.bfloat16)
    x8n = xt.astype(ml_dtypes.float8_e4m3)
    identb = np.eye(128, dtype=np.float32)
    maskL = (np.arange(128)[:, None] < np.arange(128)[None, :])
    maskL = maskL.astype(np.float32)
    maskL[127, :] = 1.0
    maskRc = np.zeros((128, 256), dtype=np.float32)
    maskRc[127, 0:128] = NEG
    for u in range(127):
        maskRc[u, 128 + u] = NEG
    maskL = maskL.astype(ml_dtypes.bfloat16)
    maskRc = maskRc.astype(ml_dtypes.bfloat16)
    in_maps = []
    for c in range(N_CORES):
        h0 = HPC * c
        wq2 = np.concatenate([Wq[h0 + i] for i in range(HPC)], axis=1)
        wk2 = np.concatenate([Wk[h0 + i] for i in range(HPC)], axis=1)
        wv2 = np.concatenate([Wv[h0 + i] for i in range(HPC)], axis=1)
        wot = np.ascontiguousarray(Wo[:, F * c:F * (c + 1)].T)
        in_maps.append({
            "xb": xbn,
            "x8": x8n,
            "wq8": (wq2 * WS).astype(ml_dtypes.float8_e4m3),
            "wk8": (wk2 * WS).astype(ml_dtypes.float8_e4m3),
            "wv": wv2.astype(ml_dtypes.bfloat16),
            "wot": wot.astype(np.float32),
            "identb": identb,
            "maskL": maskL,
            "maskRc": maskRc,
        })
    return in_maps


_NC = None
_RUN = None
DEFAULT_CFG = {}


def kernel(inputs, Wk, Wq, Wv, Wo, bo):
    global _NC, _RUN
    if _NC is None:
        _NC = build_nc(cfg=DEFAULT_CFG)
    in_maps = make_in_maps(inputs, Wk, Wq, Wv, Wo)
    try:
        if _RUN is None:
            _RUN = _make_runner(_NC)
        results = _RUN(in_maps)
    except Exception:
        _RUN = False if _RUN is None else _RUN
        res = run_bass_kernel_spmd(_NC, in_maps,
                                   core_ids=list(range(N_CORES)))
        results = res.results
    z = np.zeros((B, T, E), dtype=np.float32)
    for c in range(N_CORES):
        z += results[c]["zp"].astype(np.float32)
    return z + bo.astype(np.float32)
